# revision 16
# baseline (speedup 1.0000x reference)
"""Trainium2 Bass kernel for CInterpolateExtractor.

Pipeline: trilevel bilinear feature gather + concat + 3-layer MLP.
  latent = [pos, bilerp(f0), bilerp(f1), bilerp(f2)]  (450 ch)
  out = relu(relu(latent@W1+b1)@W2+b2)@W3 + b3        (256 ch)

Sharding: data-parallel over batch B=32 -> 4 batches per NeuronCore.

Per-core design (all layouts "natural": point n = 64*p + s within a batch):
  - one-time fp32->bf16 feature prepass into internal DRAM (halves gather bytes)
  - per batch: pos -> per-level corner indices (int32) + lerp weights (DVE)
  - per chunk of 1024 points ([128 partitions, 8 slots]):
      * 6x gpsimd.indirect_dma_start: for each level and each of y0/y1, gather
        the 2C-contiguous channel row-pair (x0,x0+1) per point -> [128, 8, 2C]
      * DVE lerp with scalar_tensor_tensor (per-partition scalar weights)
      * PE transposes of the [128, 450] latent into K-tiles [<=128, 1024]
      * L1/L2 matmuls in "B-form" (out h^T [512ch, pts]; bias+ReLU on ScalarE
        as per-partition activation bias), L3 in "A-form" (out [pts, 256];
        bias via K=1 ones-row matmul) -> contiguous 1KB/point output rows
"""

import sys

sys.path.insert(0, "/opt/trn_rl_repo")

import numpy as np

import concourse.bass as bass
import concourse.mybir as mybir
import concourse.tile as tile
from concourse import bacc
from concourse.bass import IndirectOffsetOnAxis
from concourse.bass_utils import run_bass_kernel_spmd
from concourse.masks import make_identity

F32 = mybir.dt.float32
BF16 = mybir.dt.bfloat16
I32 = mybir.dt.int32
I16 = mybir.dt.int16
AF = mybir.ActivationFunctionType
OP = mybir.AluOpType

LEVELS = [(128, 128, 64), (64, 64, 128), (32, 32, 256)]
D_LAT = 450  # 2 + 64 + 128 + 256
K_SPLITS = [(0, 128), (128, 256), (256, 384), (384, 450)]
H1D = 512
H2D = 512
OD = 256
N_CORES = 8


def build_program(b_pc: int, n_pts: int, s_chunk: int):
    """Build the per-core bass program.

    b_pc: batches per core; n_pts: points per batch; s_chunk: slots per chunk
    (chunk = 128*s_chunk points).
    """
    assert n_pts % 128 == 0
    n_slots = n_pts // 128
    assert n_slots % s_chunk == 0
    n_chunks = n_slots // s_chunk
    S = s_chunk

    nc = bacc.Bacc("TRN2", target_bir_lowering=False, debug=False,
                   num_devices=N_CORES)

    f_in = [nc.dram_tensor(f"features{i}", [b_pc, H, W, C], F32,
                           kind="ExternalInput")
            for i, (H, W, C) in enumerate(LEVELS)]
    pos_in = nc.dram_tensor("pos", [b_pc, n_pts, 2], F32, kind="ExternalInput")
    w_in = {}
    for name, shape in [("W1", [D_LAT, H1D]), ("b1", [H1D]),
                        ("W2", [H1D, H2D]), ("b2", [H2D]),
                        ("W3", [H2D, OD]), ("b3", [OD])]:
        w_in[name] = nc.dram_tensor(name, shape, F32, kind="ExternalInput")
    out_d = nc.dram_tensor("out", [b_pc * n_pts, OD], F32,
                           kind="ExternalOutput")
    # bf16 feature copies (flat); level 0 is gathered in fp32 directly
    # (bf16 row stride 64*2=128B violates dma_gather's 256B constraint)
    f_bf = [None] + [nc.dram_tensor(f"fbf{i}", [b_pc * H * W * C], BF16,
                                    kind="Internal")
                     for i, (H, W, C) in enumerate(LEVELS) if i > 0]

    with tile.TileContext(nc) as tc:
        # ---------------- phase 0: fp32 -> bf16 feature prepass -------------
        with tc.tile_pool(name="prep", bufs=3) as prep:
            for i, (H, W, C) in enumerate(LEVELS):
                if i == 0:
                    continue
                total = b_pc * H * W * C
                CH = 128 * 8192
                off = 0
                while off < total:
                    sz = min(CH, total - off)
                    assert sz % 128 == 0
                    fdim = sz // 128
                    t = prep.tile([128, fdim], BF16, tag="prep")
                    src = f_in[i].ap().rearrange("b h w c -> (b h w c)")
                    nc.gpsimd.dma_start(
                        out=t[:],
                        in_=src[off:off + sz].rearrange("(p f) -> p f", p=128))
                    dst = f_bf[i].ap()
                    nc.sync.dma_start(
                        out=dst[off:off + sz].rearrange("(p f) -> p f", p=128),
                        in_=t[:])
                    off += sz

        # ---------------- phase 1: constants / weights ----------------------
        wpool = tc.alloc_tile_pool(name="wts", bufs=1)
        stage = tc.alloc_tile_pool(name="stage", bufs=2)

        def load_w_tiles(name, rows_splits, cols, cast=True):
            tiles = []
            for t_i, (a, b) in enumerate(rows_splits):
                rows = b - a
                tf = stage.tile([rows, cols], F32, tag="wstage")
                nc.sync.dma_start(out=tf[:], in_=w_in[name].ap()[a:b, :])
                tb = wpool.tile([rows, cols], BF16, tag=f"{name}_{t_i}")
                nc.vector.tensor_copy(out=tb[:], in_=tf[:])
                tiles.append(tb)
            return tiles

        w1sb = load_w_tiles("W1", K_SPLITS, H1D)
        w2sb = load_w_tiles("W2", [(i * 128, (i + 1) * 128) for i in range(4)],
                            H2D)
        w3sb = load_w_tiles("W3", [(i * 128, (i + 1) * 128) for i in range(4)],
                            OD)

        # biases: b1/b2 as [128, 4] per-partition bias tiles (col m = M-tile m)
        bias_sb = {}
        for name, dim in [("b1", H1D), ("b2", H2D)]:
            bt = wpool.tile([128, dim // 128], F32, tag=f"{name}sb")
            nc.sync.dma_start(
                out=bt[:],
                in_=w_in[name].ap().rearrange("(m p) -> p m", p=128))
            bias_sb[name] = bt
        # b3 as a bf16 row [1, OD] for the K=1 bias matmul
        b3f = stage.tile([1, OD], F32, tag="wstage2")
        nc.sync.dma_start(out=b3f[:],
                          in_=w_in["b3"].ap().rearrange("(u o) -> u o", u=1))
        b3sb = wpool.tile([1, OD], BF16, tag="b3sb")
        nc.vector.tensor_copy(out=b3sb[:], in_=b3f[:])

        ones_row = wpool.tile([1, 128], BF16, tag="ones")
        nc.gpsimd.memset(ones_row[:], 1.0)
        idsb = wpool.tile([128, 128], BF16, tag="ident")
        make_identity(nc, idsb[:])

        # ---------------- phase 2: main loop --------------------------------
        ppool = tc.alloc_tile_pool(name="posidx", bufs=2)
        gpool = tc.alloc_tile_pool(name="gather", bufs=2)
        vpool = tc.alloc_tile_pool(name="vlat", bufs=2)
        ltpool = tc.alloc_tile_pool(name="latT", bufs=2)
        hpool = tc.alloc_tile_pool(name="hact", bufs=1)
        opool = tc.alloc_tile_pool(name="outb", bufs=2)
        pstp = tc.alloc_tile_pool(name="ps_tp", bufs=2, space="PSUM")
        psmm = tc.alloc_tile_pool(name="ps_mm", bufs=2, space="PSUM")
        psl3 = tc.alloc_tile_pool(name="ps_l3", bufs=2, space="PSUM")

        out_v = out_d.ap().rearrange("(b p s) o -> b p s o", b=b_pc, p=128,
                                     s=n_slots)

        GP = 8  # partition groups of 16
        w_base = b_pc * 16

        # ---- wrapped pos load + wrapped int16 index computation ------------
        # dma_gather consumes indices "wrapped in 16 partitions": list
        # position i lives at idx tile [i%16, i//16], replicated across the
        # 8 Q7 core groups. Gather position i = s*128 + p lands at out
        # partition p, slot s, and must carry the index of point n = 64p+s,
        # so the wrapped tile is idxw[r, 8s + g] = idx_nat[16g + r, s].
        # Load pos a second time in this wrapped layout (batch b on
        # partitions 16b:16b+16, free dims (g, s, c)) and compute indices
        # there; the final cast-copy reorders (g, s) -> (s, g).
        PW = ppool.tile([w_base, GP, n_slots, 2], F32, tag="pw", bufs=1)
        for b in range(b_pc):
            src = pos_in.ap().rearrange("b n c -> (b n c)")[
                b * n_pts * 2:].copy()
            src.ap = mybir.VecI64Pair([
                (n_slots * 2, 16), (16 * n_slots * 2, GP), (2, n_slots),
                (1, 2)])
            nc.sync.dma_start(out=PW[16 * b:16 * (b + 1)], in_=src)

        idxw_all = ppool.tile([w_base, 6, n_slots, GP], I16, tag="idxw",
                              bufs=1)
        for li, (H, W, C) in enumerate(LEVELS):
            swc = ppool.tile([w_base, GP, n_slots, 2], F32, tag="wsc", bufs=1)
            nc.vector.tensor_scalar_mul(swc[:, :, :, 0], PW[:, :, :, 0],
                                        float(H - 1))
            nc.vector.tensor_scalar_mul(swc[:, :, :, 1], PW[:, :, :, 1],
                                        float(W - 1))
            wci = ppool.tile([w_base, GP, n_slots, 2], I32, tag="wci", bufs=1)
            nc.vector.tensor_copy(out=wci[:], in_=swc[:])
            wcf = ppool.tile([w_base, GP, n_slots, 2], F32, tag="wcf", bufs=1)
            nc.vector.tensor_copy(out=wcf[:], in_=wci[:])
            wadj = ppool.tile([w_base, GP, n_slots, 2], F32, tag="wadj", bufs=1)
            nc.vector.tensor_tensor(out=wadj[:], in0=wcf[:], in1=swc[:],
                                    op=OP.is_gt)
            nc.vector.tensor_tensor(out=wcf[:], in0=wcf[:], in1=wadj[:],
                                    op=OP.subtract)
            nc.vector.tensor_scalar(wcf[:, :, :, 0], wcf[:, :, :, 0],
                                    float(H - 2), None, op0=OP.min)
            nc.vector.tensor_scalar(wcf[:, :, :, 1], wcf[:, :, :, 1],
                                    float(W - 2), None, op0=OP.min)
            i0f = ppool.tile([w_base, GP, n_slots], F32, tag="wi0f", bufs=1)
            nc.vector.scalar_tensor_tensor(
                out=i0f[:], in0=wcf[:, :, :, 0], scalar=float(W),
                in1=wcf[:, :, :, 1], op0=OP.mult, op1=OP.add)
            nc.vector.tensor_copy(
                out=idxw_all[:, 2 * li],
                in_=i0f[:].rearrange("p g s -> p s g"))
            nc.vector.tensor_scalar_add(i0f[:], i0f[:], float(W))
            nc.vector.tensor_copy(
                out=idxw_all[:, 2 * li + 1],
                in_=i0f[:].rearrange("p g s -> p s g"))

        for b in range(b_pc):
            # replicate this batch's wrapped indices to all 8 core groups
            idxw = ppool.tile([128, 6, n_slots, GP], I16, tag="idxwrep")
            for g in range(GP):
                nc.sync.dma_start(out=idxw[16 * g:16 * (g + 1)],
                                  in_=idxw_all[16 * b:16 * (b + 1)])

            # ---- pos load (natural layout: point n=64p+s -> [p, s]) --------
            P = ppool.tile([128, n_slots, 2], F32, tag="pos")
            nc.sync.dma_start(
                out=P[:],
                in_=pos_in.ap()[b].rearrange("(p s) c -> p s c", p=128))

            wf_t = []    # fp32 frac weights [128, n_slots, 2] (y, x)
            wy_t = []    # bf16 [128, n_slots]
            wx_t = []
            for li, (H, W, C) in enumerate(LEVELS):
                sc = ppool.tile([128, n_slots, 2], F32, tag=f"sc{li}", bufs=1)
                nc.vector.tensor_scalar_mul(sc[:, :, 0], P[:, :, 0],
                                            float(H - 1))
                nc.vector.tensor_scalar_mul(sc[:, :, 1], P[:, :, 1],
                                            float(W - 1))
                # floor(sc) robust to the f32->i32 cast rounding mode:
                # cf = f32(i32(sc)); c0 = cf - (cf > sc)
                ci = ppool.tile([128, n_slots, 2], I32, tag=f"ci{li}", bufs=1)
                nc.vector.tensor_copy(out=ci[:], in_=sc[:])
                cf = ppool.tile([128, n_slots, 2], F32, tag=f"cf{li}", bufs=1)
                nc.vector.tensor_copy(out=cf[:], in_=ci[:])
                adj = ppool.tile([128, n_slots, 2], F32, tag=f"adj{li}", bufs=1)
                nc.vector.tensor_tensor(out=adj[:], in0=cf[:], in1=sc[:],
                                        op=OP.is_gt)
                c0 = ppool.tile([128, n_slots, 2], F32, tag=f"c0{li}", bufs=1)
                nc.vector.tensor_tensor(out=c0[:], in0=cf[:], in1=adj[:],
                                        op=OP.subtract)
                nc.vector.tensor_scalar(c0[:, :, 0], c0[:, :, 0],
                                        float(H - 2), None, op0=OP.min)
                nc.vector.tensor_scalar(c0[:, :, 1], c0[:, :, 1],
                                        float(W - 2), None, op0=OP.min)
                wf = ppool.tile([128, n_slots, 2], F32, tag=f"wf{li}")
                nc.vector.tensor_tensor(out=wf[:], in0=sc[:], in1=c0[:],
                                        op=OP.subtract)
                wf_t.append(wf)
                wyb = ppool.tile([128, n_slots], BF16, tag=f"wy{li}")
                wxb = ppool.tile([128, n_slots], BF16, tag=f"wx{li}")
                nc.vector.tensor_copy(out=wyb[:], in_=wf[:, :, 0])
                nc.vector.tensor_copy(out=wxb[:], in_=wf[:, :, 1])
                wy_t.append(wyb)
                wx_t.append(wxb)

            for c in range(n_chunks):
                s0 = c * S
                # ---- gathers: 2C-contiguous row pairs per point ------------
                G = []
                for li, (H, W, C) in enumerate(LEVELS):
                    if li == 0:
                        gdt = F32
                        base = f_in[0].ap().rearrange("b h w c -> (b h w c)")
                    else:
                        gdt = BF16
                        base = f_bf[li].ap()
                    rows = []
                    for r in range(2):
                        g = gpool.tile([128, S, 2 * C], gdt,
                                       tag=f"g{li}_{r}", name=f"g{li}_{r}")
                        in_ap = base[b * H * W * C:].copy()
                        in_ap.ap = mybir.VecI64Pair(
                            [(C, H * W - 1), (1, 2 * C)])
                        nc.gpsimd.dma_gather(
                            out_ap=g[:], in_ap=in_ap,
                            idxs_ap=idxw[:, 2 * li + r, s0:s0 + S, :],
                            num_idxs=128 * S, num_idxs_reg=128 * S,
                            elem_size=2 * C, elem_step=C)
                        rows.append(g)
                    G.append(rows)

                # ---- lerp -> V [128, S, 450] bf16 --------------------------
                V = vpool.tile([128, S, D_LAT], BF16, tag="v")
                nc.vector.tensor_copy(out=V[:, :, 0:2],
                                      in_=P[:, s0:s0 + S, :])
                off = 2
                for li, (H, W, C) in enumerate(LEVELS):
                    g0, g1 = G[li]
                    # level 0 is gathered fp32 -> use fp32 scalar weights
                    if li == 0:
                        wy_s = lambda k: wf_t[0][:, s0 + k, 0:1]
                        wx_s = lambda k: wf_t[0][:, s0 + k, 1:2]
                    else:
                        wy_s = lambda k, t=wy_t[li]: t[:, s0 + k:s0 + k + 1]
                        wx_s = lambda k, t=wx_t[li]: t[:, s0 + k:s0 + k + 1]
                    # g1 <- g1 - g0 (in place)
                    nc.vector.tensor_tensor(out=g1[:], in0=g1[:], in1=g0[:],
                                            op=OP.subtract)
                    for k in range(S):
                        # g0[k] <- g1[k]*wy + g0[k]  (y-lerp, in place)
                        nc.vector.scalar_tensor_tensor(
                            out=g0[:, k, :], in0=g1[:, k, :],
                            scalar=wy_s(k),
                            in1=g0[:, k, :], op0=OP.mult, op1=OP.add)
                    # dx = r[x1] - r[x0] (into g1 low half)
                    nc.vector.tensor_tensor(out=g1[:, :, 0:C],
                                            in0=g0[:, :, C:2 * C],
                                            in1=g0[:, :, 0:C],
                                            op=OP.subtract)
                    for k in range(S):
                        nc.vector.scalar_tensor_tensor(
                            out=V[:, k, off:off + C], in0=g1[:, k, 0:C],
                            scalar=wx_s(k),
                            in1=g0[:, k, 0:C], op0=OP.mult, op1=OP.add)
                    off += C
                assert off == D_LAT

                # ---- latent transpose: LT[t] [rows_t, S*128] bf16 ----------
                LT = [ltpool.tile([128, S * 128], BF16, tag=f"lt{t}",
                                  name=f"lt{t}")
                      for t in range(4)]
                for k in range(S):
                    for t, (a, bnd) in enumerate(K_SPLITS):
                        rows = bnd - a
                        tp = pstp.tile([128, 128], BF16, tag="tp",
                                       space="PSUM")
                        nc.tensor.transpose(out=tp[:rows, :],
                                            in_=V[:, k, a:bnd],
                                            identity=idsb[:])
                        dst = LT[t][:rows, k * 128:(k + 1) * 128]
                        if (k * 4 + t) % 2 == 0:
                            nc.scalar.activation(out=dst, in_=tp[:rows, :],
                                                 func=AF.Copy)
                        else:
                            nc.vector.tensor_copy(out=dst, in_=tp[:rows, :])

                # ---- L1 / L2 (B-form) --------------------------------------
                def mlp_layer(w_tiles, rhs_tiles, bias_tile, n_out, htag):
                    m_tiles = n_out // 128
                    h = [hpool.tile([128, S * 128], BF16, tag=f"{htag}{m}",
                                    name=f"{htag}{m}")
                         for m in range(m_tiles)]
                    for n in range((S * 128) // 512):
                        nsl = slice(n * 512, (n + 1) * 512)
                        for m in range(m_tiles):
                            ps = psmm.tile([128, 512], F32, tag="mm",
                                           space="PSUM")
                            for t, wt in enumerate(w_tiles):
                                rows = wt.shape[0]
                                nc.tensor.matmul(
                                    out=ps[:],
                                    lhsT=wt[:, m * 128:(m + 1) * 128],
                                    rhs=rhs_tiles[t][:rows, nsl],
                                    start=(t == 0),
                                    stop=(t == len(w_tiles) - 1))
                            nc.scalar.activation(
                                out=h[m][:, nsl], in_=ps[:], func=AF.Relu,
                                bias=bias_tile[:, m:m + 1])
                    return h

                h1 = mlp_layer(w1sb, LT, bias_sb["b1"], H1D, "h1_")
                h2 = mlp_layer(w2sb, h1, bias_sb["b2"], H2D, "h2_")

                # ---- L3 (A-form): out[pts,256] = h2T.T @ W3 + b3 -----------
                O = opool.tile([128, S, OD], F32, tag="obuf")
                for j in range(S):
                    ps3 = psl3.tile([128, OD], F32, tag="l3", space="PSUM")
                    for t in range(4):
                        nc.tensor.matmul(
                            out=ps3[:],
                            lhsT=h2[t][:, j * 128:(j + 1) * 128],
                            rhs=w3sb[t][:],
                            start=(t == 0), stop=False)
                    nc.tensor.matmul(out=ps3[:], lhsT=ones_row[:],
                                     rhs=b3sb[:], start=False, stop=True)
                    if j % 2 == 0:
                        nc.scalar.activation(out=O[:, j, :], in_=ps3[:],
                                             func=AF.Copy)
                    else:
                        nc.vector.tensor_copy(out=O[:, j, :], in_=ps3[:])

                nc.sync.dma_start(out=out_v[b, :, s0:s0 + S, :], in_=O[:])

        for _pool in (psl3, psmm, pstp, opool, hpool, ltpool, vpool, gpool,
                      ppool, stage, wpool):
            _pool.release()

    nc.compile()
    return nc


_CACHE = {}


def _get_program():
    key = (4, 8192, 8)
    if key not in _CACHE:
        _CACHE[key] = build_program(*key)
    return _CACHE[key]


def kernel(features0, features1, features2, pos, W1, b1, W2, b2, W3, b3):
    nc = _get_program()
    B = pos.shape[0]
    bpc = B // N_CORES
    in_maps = []
    for i in range(N_CORES):
        sl = slice(i * bpc, (i + 1) * bpc)
        in_maps.append({
            "features0": np.ascontiguousarray(features0[sl]),
            "features1": np.ascontiguousarray(features1[sl]),
            "features2": np.ascontiguousarray(features2[sl]),
            "pos": np.ascontiguousarray(pos[sl]),
            "W1": np.asarray(W1), "b1": np.asarray(b1),
            "W2": np.asarray(W2), "b2": np.asarray(b2),
            "W3": np.asarray(W3), "b3": np.asarray(b3),
        })
    res = run_bass_kernel_spmd(nc, in_maps, core_ids=list(range(N_CORES)))
    return np.concatenate([res.results[i]["out"] for i in range(N_CORES)],
                          axis=0)
